# revision 2
# baseline (speedup 1.0000x reference)
"""Trainium2 Bass kernel for CURLoRA forward: out = x @ (C @ U @ R).T

Fused low-rank chain per core: t1.T = sum_k R_k.T.T @ x_k.T  (K-tiled over m),
t2.T = U.T.T @ t1.T, out = t2.T.T @ C_shard.T — never materializes W_approx.

Sharding (8 cores): s-dim (128 rows of x) split 4 ways x n-dim (8192 out cols)
split 2 ways. Per core DMA: 1MB x-shard + 2MB R (replicated) + 1MB C.T shard
+ 0.5MB out. All transposes are host-side layout prep during sharding.
"""

import numpy as np

B, S, M, N, RANK = 2, 64, 8192, 8192, 64
NCORES = 8
SA, NB = 4, 2              # s-blocks x n-blocks = 8 cores
SSH = (B * S) // SA        # 32 s-rows per core
NSH = N // NB              # 4096 out cols per core
KCH = M // 128             # 64 contraction chunks of 128

_NC_CACHE = {}


def _build_nc():
    if "nc" in _NC_CACHE:
        return _NC_CACHE["nc"]
    from concourse import bacc, mybir
    import concourse.bass as bass
    from concourse.tile import TileContext

    f32 = mybir.dt.float32
    nc = bacc.Bacc(None)

    xp_d = nc.declare_dram_parameter("xp", [128, KCH * SSH], f32, isOutput=False)
    rp_d = nc.declare_dram_parameter("rp", [128, KCH * RANK], f32, isOutput=False)
    ut_d = nc.declare_dram_parameter("ut", [RANK, RANK], f32, isOutput=False)
    ct_d = nc.declare_dram_parameter("ct", [RANK, NSH], f32, isOutput=False)
    out_d = nc.declare_dram_parameter("out", [128, NSH // 4], f32, isOutput=True)

    NP = 4                 # input DMA pieces for pipelining
    KPP = KCH // NP        # k-chunks per piece

    with TileContext(nc) as tc:
        with (
            tc.tile_pool(name="sb", bufs=1) as sb,
            tc.tile_pool(name="ps", bufs=1, space=bass.MemorySpace.PSUM) as ps,
        ):
            xts, rts = [], []
            for p in range(NP):
                xt = sb.tile([128, KPP * SSH], f32, tag=f"x{p}")
                rt = sb.tile([128, KPP * RANK], f32, tag=f"r{p}")
                nc.sync.dma_start(xt[:], xp_d[:, p * KPP * SSH:(p + 1) * KPP * SSH])
                nc.sync.dma_start(rt[:], rp_d[:, p * KPP * RANK:(p + 1) * KPP * RANK])
                xts.append(xt)
                rts.append(rt)
            utt = sb.tile([RANK, RANK], f32, tag="ut")
            nc.sync.dma_start(utt[:], ut_d[:])
            cts = []
            for h in range(2):
                ct = sb.tile([RANK, NSH // 2], f32, tag=f"c{h}")
                nc.sync.dma_start(ct[:], ct_d[:, h * (NSH // 2):(h + 1) * (NSH // 2)])
                cts.append(ct)

            # stage 1: t1.T [64, 32] accumulated over 64 k-chunks
            psum1 = ps.tile([RANK, SSH], f32, tag="p1")
            for k in range(KCH):
                p, kl = divmod(k, KPP)
                nc.tensor.matmul(
                    psum1[:],
                    rts[p][:, kl * RANK:(kl + 1) * RANK],
                    xts[p][:, kl * SSH:(kl + 1) * SSH],
                    start=(k == 0),
                    stop=(k == KCH - 1),
                )
            t1s = sb.tile([RANK, SSH], f32, tag="t1")
            nc.vector.tensor_copy(t1s[:], psum1[:])

            # stage 2: t2.T = U.T.T @ t1.T
            psum2 = ps.tile([RANK, SSH], f32, tag="p2")
            nc.tensor.matmul(psum2[:], utt[:], t1s[:], start=True, stop=True)
            t2s = sb.tile([RANK, SSH], f32, tag="t2")
            nc.vector.tensor_copy(t2s[:], psum2[:])

            # stage 3: out[s, n] col-tiled into [128, 1024] psum (4 s-quarters
            # stacked on partitions, 2 col-halves = 2 banks)
            pso = ps.tile([128, 2 * 512], f32, tag="po")
            osb = sb.tile([128, 2 * 512], f32, tag="osb")
            for jj in range(8):
                q, h = jj % 4, jj // 4
                nc.tensor.matmul(
                    pso[q * SSH:(q + 1) * SSH, h * 512:(h + 1) * 512],
                    t2s[:],
                    cts[h][:, q * 512:(q + 1) * 512],
                    start=True,
                    stop=True,
                    tile_position=(0, q * SSH),
                )
            for h in range(2):
                nc.vector.tensor_copy(
                    osb[:, h * 512:(h + 1) * 512], pso[:, h * 512:(h + 1) * 512]
                )
            nc.sync.dma_start(out_d[:], osb[:])

    nc.compile()
    _NC_CACHE["nc"] = nc
    return nc


def _shard_inputs(x, C, U, R):
    xf = np.asarray(x, np.float32).reshape(B * S, M)
    C = np.asarray(C, np.float32)
    U = np.asarray(U, np.float32)
    R = np.asarray(R, np.float32)

    # rp[p, k*64+r] = R[r, 128k+p]
    rp = np.ascontiguousarray(
        R.reshape(RANK, KCH, 128).transpose(2, 1, 0)
    ).reshape(128, KCH * RANK)
    ut = np.ascontiguousarray(U.T)

    in_maps = []
    for c in range(NCORES):
        i, j = divmod(c, NB)
        xs = xf[i * SSH:(i + 1) * SSH, :]
        # xp[p, k*32+s] = xs[s, 128k+p]
        xp = np.ascontiguousarray(
            xs.reshape(SSH, KCH, 128).transpose(2, 1, 0)
        ).reshape(128, KCH * SSH)
        ct = np.ascontiguousarray(C[j * NSH:(j + 1) * NSH, :].T)
        in_maps.append({"xp": xp, "rp": rp, "ut": ut, "ct": ct})
    return in_maps


def _unshard_output(core_outs):
    full = np.empty((B * S, N), np.float32)
    for c in range(NCORES):
        i, j = divmod(c, NB)
        q = core_outs[c]  # [128, 1024]: q[32a+s, 512h+nr] = out[s, (4h+a)*512+nr]
        blk = q.reshape(4, SSH, 2, 512).transpose(1, 2, 0, 3).reshape(SSH, NSH)
        full[i * SSH:(i + 1) * SSH, j * NSH:(j + 1) * NSH] = blk
    return full.reshape(B, S, N)


def _ensure_ntff_hook():
    """bass_utils' axon trace path imports antenv.axon_hooks, which this
    container's antenv lacks. Register an equivalent module backed by the
    boot package's ctypes NTFF hook so trace=True (or BASS_TRACE=1) works."""
    import sys
    import types

    try:
        from antenv.axon_hooks import get_axon_ntff_profile_hook  # noqa: F401
        return
    except ImportError:
        pass
    try:
        from trn_agent_boot.trn_boot import _ntff_profile_via_ctypes

        hook = _ntff_profile_via_ctypes("/opt/axon/libaxon_pjrt.so")
    except Exception:
        hook = None
    mod = types.ModuleType("antenv.axon_hooks")
    state = {"hook": hook}
    mod.get_axon_ntff_profile_hook = lambda: state["hook"]
    mod.set_axon_ntff_profile_hook = lambda h: state.update(hook=h)
    sys.modules["antenv.axon_hooks"] = mod


def run(x, C, U, R, trace=False, **spmd_kwargs):
    from concourse.bass_utils import run_bass_kernel_spmd

    _ensure_ntff_hook()
    nc = _build_nc()
    in_maps = _shard_inputs(x, C, U, R)
    res = run_bass_kernel_spmd(
        nc, in_maps, core_ids=list(range(NCORES)), trace=trace, **spmd_kwargs
    )
    out = _unshard_output([r["out"] for r in res.results])
    return out, res


def kernel(x, C, U, R):
    out, _ = run(x, C, U, R, trace=False)
    return out


# revision 3
# speedup vs baseline: 1.1116x; 1.1116x over previous
"""Trainium2 Bass kernel for CURLoRA forward: out = x @ (C @ U @ R).T

Fused low-rank chain per core (never materializes W_approx):
  t1.T = sum_k R_k.T.T @ x_k.T   (64 K-tiles of 128, PE col-group ping-pong)
  t2.T = [U.T;U.T].T @ [t1a;t1b] (K=128 contraction absorbs the col-group sum)
  out  = col-tiled t2.T.T @ C.T  into one [128,1024] PSUM block

Sharding (8 cores): s-dim (128 rows of x) split 4 ways x n-dim (8192 out
cols) split 2 ways. Per core DMA: 1MB x-shard + 2MB R (replicated) + 1MB
C.T shard + 0.5MB out. All transposes are host-side layout prep.
"""

import numpy as np

B, S, M, N, RANK = 2, 64, 8192, 8192, 64
NCORES = 8
SA, NB = 4, 2              # s-blocks x n-blocks = 8 cores
SSH = (B * S) // SA        # 32 s-rows per core
NSH = N // NB              # 4096 out cols per core
KCH = M // 128             # 64 contraction chunks of 128

_NC_CACHE = {}


def _build_nc():
    if "nc" in _NC_CACHE:
        return _NC_CACHE["nc"]
    from concourse import bacc, mybir
    import concourse.bass as bass
    from concourse.tile import TileContext

    f32 = mybir.dt.float32
    nc = bacc.Bacc(None)

    xp_d = nc.declare_dram_parameter("xp", [128, KCH * SSH], f32, isOutput=False)
    rp_d = nc.declare_dram_parameter("rp", [128, KCH * RANK], f32, isOutput=False)
    u2_d = nc.declare_dram_parameter("u2", [128, RANK], f32, isOutput=False)
    ct_d = nc.declare_dram_parameter("ct", [128, NSH // 2], f32, isOutput=False)
    out_d = nc.declare_dram_parameter("out", [128, NSH // 4], f32, isOutput=True)

    NP = 4                 # x/R DMA pieces for pipelining
    KPP = KCH // NP        # k-chunks per piece

    with TileContext(nc) as tc:
        with (
            tc.tile_pool(name="sb", bufs=1) as sb,
            tc.tile_pool(name="ps", bufs=1, space=bass.MemorySpace.PSUM) as ps,
        ):
            # inputs; descriptor-gen split across sync/scalar/gpsimd
            xts, rts = [], []
            for p in range(NP):
                xt = sb.tile([128, KPP * SSH], f32, tag=f"x{p}")
                rt = sb.tile([128, KPP * RANK], f32, tag=f"r{p}")
                nc.sync.dma_start(xt[:], xp_d[:, p * KPP * SSH:(p + 1) * KPP * SSH])
                nc.scalar.dma_start(rt[:], rp_d[:, p * KPP * RANK:(p + 1) * KPP * RANK])
                xts.append(xt)
                rts.append(rt)
            u2t = sb.tile([128, RANK], f32, tag="u2")
            nc.gpsimd.dma_start(u2t[:], u2_d[:])
            cts = []
            for p in range(2):
                ct = sb.tile([128, 1024], f32, tag=f"c{p}")
                nc.gpsimd.dma_start(ct[:], ct_d[:, p * 1024:(p + 1) * 1024])
                cts.append(ct)

            # stage 1: t1.T [64, 32] x2 via col-group ping-pong, one PSUM
            # bank per group so accumulation flags don't collide
            ps1a = ps.tile([64, SSH], f32, tag="p1a")
            ps1b = ps.tile([128, SSH], f32, tag="p1b")
            for k in range(KCH):
                p, kl = divmod(k, KPP)
                lhsT = rts[p][:, kl * RANK:(kl + 1) * RANK]
                rhs = xts[p][:, kl * SSH:(kl + 1) * SSH]
                if k % 2 == 0:
                    nc.tensor.matmul(
                        ps1a[:], lhsT, rhs,
                        start=(k == 0), stop=(k == KCH - 2),
                        tile_position=(0, 0),
                    )
                else:
                    nc.tensor.matmul(
                        ps1b[64:128, :], lhsT, rhs,
                        start=(k == 1), stop=(k == KCH - 1),
                        tile_position=(0, 64),
                    )
            t1s = sb.tile([128, SSH], f32, tag="t1")
            nc.vector.tensor_copy(t1s[0:64, :], ps1a[:])
            nc.vector.tensor_copy(t1s[64:128, :], ps1b[64:128, :])

            # stage 2: t2.T = [U.T;U.T].T @ t1s (K=128 sums the two groups);
            # duplicated into both partition halves for stage 3's row halves
            ps2 = ps.tile([128, SSH], f32, tag="p2")
            nc.tensor.matmul(ps2[0:64, :], u2t[:], t1s[:],
                             start=True, stop=True, tile_position=(0, 0))
            nc.tensor.matmul(ps2[64:128, :], u2t[:], t1s[:],
                             start=True, stop=True, tile_position=(0, 64))
            t2s = sb.tile([128, SSH], f32, tag="t2")
            nc.vector.tensor_copy(t2s[:], ps2[:])

            # stage 3: out[s, n] col-tiled into [128, 1024] psum (4 s-quarters
            # on partitions x 2 col-halves = 2 banks); ct piece p holds n-cols
            # [p*1024,(p+1)*1024) in rows 0:64 and [2048+p*1024, ...) in 64:128
            pso = ps.tile([128, 2 * 512], f32, tag="po")
            osb = sb.tile([128, 2 * 512], f32, tag="osb")
            for rh in range(2):          # row half of ct = psum col half
                for p in range(2):       # ct piece
                    for w in range(2):   # 512-block within piece
                        q = p * 2 + w    # psum partition quarter
                        nc.tensor.matmul(
                            pso[q * SSH:(q + 1) * SSH, rh * 512:(rh + 1) * 512],
                            t2s[rh * 64:(rh + 1) * 64, :],
                            cts[p][rh * 64:(rh + 1) * 64, w * 512:(w + 1) * 512],
                            start=True, stop=True,
                            tile_position=(rh * 64, q * SSH),
                        )
            for h in range(2):
                nc.vector.tensor_copy(
                    osb[:, h * 512:(h + 1) * 512], pso[:, h * 512:(h + 1) * 512]
                )
            nc.sync.dma_start(out_d[:], osb[:])

    nc.compile()
    _NC_CACHE["nc"] = nc
    return nc


def _shard_inputs(x, C, U, R):
    xf = np.asarray(x, np.float32).reshape(B * S, M)
    C = np.asarray(C, np.float32)
    U = np.asarray(U, np.float32)
    R = np.asarray(R, np.float32)

    # rp[p, k*64+r] = R[r, 128k+p]
    rp = np.ascontiguousarray(
        R.reshape(RANK, KCH, 128).transpose(2, 1, 0)
    ).reshape(128, KCH * RANK)
    u2 = np.ascontiguousarray(np.concatenate([U.T, U.T], axis=0))

    in_maps = []
    for c in range(NCORES):
        i, j = divmod(c, NB)
        xs = xf[i * SSH:(i + 1) * SSH, :]
        # xp[p, k*32+s] = xs[s, 128k+p]
        xp = np.ascontiguousarray(
            xs.reshape(SSH, KCH, 128).transpose(2, 1, 0)
        ).reshape(128, KCH * SSH)
        cT = C[j * NSH:(j + 1) * NSH, :].T  # [64, 4096]
        ct = np.ascontiguousarray(
            np.concatenate([cT[:, :2048], cT[:, 2048:]], axis=0)
        )  # [128, 2048]
        in_maps.append({"xp": xp, "rp": rp, "u2": u2, "ct": ct})
    return in_maps


def _unshard_output(core_outs):
    full = np.empty((B * S, N), np.float32)
    for c in range(NCORES):
        i, j = divmod(c, NB)
        q = core_outs[c]  # [128, 1024]: q[32a+s, 512h+nr] = out[s, (4h+a)*512+nr]
        blk = q.reshape(4, SSH, 2, 512).transpose(1, 2, 0, 3).reshape(SSH, NSH)
        full[i * SSH:(i + 1) * SSH, j * NSH:(j + 1) * NSH] = blk
    return full.reshape(B, S, N)


def _ensure_ntff_hook():
    """bass_utils' axon trace path imports antenv.axon_hooks, which this
    container's antenv lacks. Register an equivalent module backed by the
    boot package's ctypes NTFF hook so trace=True (or BASS_TRACE=1) works."""
    import sys
    import types

    try:
        from antenv.axon_hooks import get_axon_ntff_profile_hook  # noqa: F401
        return
    except ImportError:
        pass
    try:
        from trn_agent_boot.trn_boot import _ntff_profile_via_ctypes

        hook = _ntff_profile_via_ctypes("/opt/axon/libaxon_pjrt.so")
    except Exception:
        hook = None
    mod = types.ModuleType("antenv.axon_hooks")
    state = {"hook": hook}
    mod.get_axon_ntff_profile_hook = lambda: state["hook"]
    mod.set_axon_ntff_profile_hook = lambda h: state.update(hook=h)
    sys.modules["antenv.axon_hooks"] = mod


def run(x, C, U, R, trace=False, **spmd_kwargs):
    from concourse.bass_utils import run_bass_kernel_spmd

    _ensure_ntff_hook()
    nc = _build_nc()
    in_maps = _shard_inputs(x, C, U, R)
    res = run_bass_kernel_spmd(
        nc, in_maps, core_ids=list(range(NCORES)), trace=trace, **spmd_kwargs
    )
    out = _unshard_output([r["out"] for r in res.results])
    return out, res


def kernel(x, C, U, R):
    out, _ = run(x, C, U, R, trace=False)
    return out


# revision 4
# speedup vs baseline: 1.1533x; 1.0375x over previous
"""Trainium2 Bass kernel for CURLoRA forward: out = x @ (C @ U @ R).T

Fused low-rank chain per core (never materializes W_approx):
  t1.T = sum_k R_k.T.T @ x_k.T   (64 K-tiles of 128, PE col-group ping-pong)
  t2.T = [U.T;U.T].T @ [t1a;t1b] (K=128 contraction absorbs the col-group sum)
  out  = col-tiled t2.T.T @ C.T  into [128,512] PSUM banks, pipelined out

Sharding (8 cores): s-dim (128 rows of x) split 4 ways x n-dim (8192 out
cols) split 2 ways. Per core DMA: 1MB x-shard + 2MB R (replicated) + 1MB
C.T shard + 0.5MB out. All transposes are host-side layout prep.
"""

import numpy as np

B, S, M, N, RANK = 2, 64, 8192, 8192, 64
NCORES = 8
SA, NB = 4, 2              # s-blocks x n-blocks = 8 cores
SSH = (B * S) // SA        # 32 s-rows per core
NSH = N // NB              # 4096 out cols per core
KCH = M // 128             # 64 contraction chunks of 128

# k-chunks per DMA piece: small first (early PE start), small last (short
# PE chase after the final bytes land)
PIECES = (8, 24, 24, 8)

_NC_CACHE = {}


def _build_nc():
    if "nc" in _NC_CACHE:
        return _NC_CACHE["nc"]
    from concourse import bacc, mybir
    import concourse.bass as bass
    from concourse.tile import TileContext, add_dep_helper

    f32 = mybir.dt.float32
    nc = bacc.Bacc(None)

    xp_d = nc.declare_dram_parameter("xp", [128, KCH * SSH], f32, isOutput=False)
    rp_d = nc.declare_dram_parameter("rp", [128, KCH * RANK], f32, isOutput=False)
    u2_d = nc.declare_dram_parameter("u2", [128, RANK], f32, isOutput=False)
    ct_d = nc.declare_dram_parameter("ct", [128, NSH // 2], f32, isOutput=False)
    out_d = nc.declare_dram_parameter("out", [128, NSH // 4], f32, isOutput=True)

    with TileContext(nc) as tc:
        with (
            tc.tile_pool(name="sb", bufs=1) as sb,
            tc.tile_pool(name="ps", bufs=1, space=bass.MemorySpace.PSUM) as ps,
        ):
            # x/R input stream; descriptor-gen split across sync/scalar
            xts, rts, xdmas, rdmas = [], [], [], []
            off = 0
            for p, kw in enumerate(PIECES):
                xt = sb.tile([128, kw * SSH], f32, tag=f"x{p}")
                rt = sb.tile([128, kw * RANK], f32, tag=f"r{p}")
                xdmas.append(
                    nc.sync.dma_start(xt[:], xp_d[:, off * SSH:(off + kw) * SSH])
                )
                rdmas.append(
                    nc.scalar.dma_start(rt[:], rp_d[:, off * RANK:(off + kw) * RANK])
                )
                xts.append(xt)
                rts.append(rt)
                off += kw
            u2t = sb.tile([128, RANK], f32, tag="u2")
            nc.gpsimd.dma_start(u2t[:], u2_d[:])
            # ct piece p holds n-cols [p*1024,(p+1)*1024) in rows 0:64 and
            # [2048+p*1024, ...) in rows 64:128. Gated mid-x-stream so the
            # critical x/R stream keeps most of the HBM bandwidth.
            cts = []
            for p in range(2):
                ct = sb.tile([128, 1024], f32, tag=f"c{p}")
                d = nc.gpsimd.dma_start(ct[:], ct_d[:, p * 1024:(p + 1) * 1024])
                add_dep_helper(
                    d.ins, (xdmas[1] if p == 0 else rdmas[1]).ins,
                    sync=True, reason="ct yields HBM bw to x/R stream",
                )
                cts.append(ct)

            # stage 1: t1.T [64, 32] x2 via col-group ping-pong, one PSUM
            # bank per group so accumulation flags don't collide
            ps1a = ps.tile([64, SSH], f32, tag="p1a")
            ps1b = ps.tile([128, SSH], f32, tag="p1b")
            k = 0
            for p, kw in enumerate(PIECES):
                for kl in range(kw):
                    lhsT = rts[p][:, kl * RANK:(kl + 1) * RANK]
                    rhs = xts[p][:, kl * SSH:(kl + 1) * SSH]
                    if k % 2 == 0:
                        nc.tensor.matmul(
                            ps1a[:], lhsT, rhs,
                            start=(k == 0), stop=(k == KCH - 2),
                            tile_position=(0, 0),
                        )
                    else:
                        nc.tensor.matmul(
                            ps1b[64:128, :], lhsT, rhs,
                            start=(k == 1), stop=(k == KCH - 1),
                            tile_position=(0, 64),
                        )
                    k += 1
            t1s = sb.tile([128, SSH], f32, tag="t1")
            nc.vector.tensor_copy(t1s[0:64, :], ps1a[:])
            nc.vector.tensor_copy(t1s[64:128, :], ps1b[64:128, :])

            # stage 2: t2.T = [U.T;U.T].T @ t1s (K=128 sums the two groups);
            # duplicated into both partition halves for stage 3's row halves
            ps2 = ps.tile([128, SSH], f32, tag="p2")
            nc.tensor.matmul(ps2[0:64, :], u2t[:], t1s[:],
                             start=True, stop=True, tile_position=(0, 0))
            nc.tensor.matmul(ps2[64:128, :], u2t[:], t1s[:],
                             start=True, stop=True, tile_position=(0, 64))
            t2s = sb.tile([128, SSH], f32, tag="t2")
            nc.vector.tensor_copy(t2s[:], ps2[:])

            # stage 3: out[s, n] col-tiled, 4 s-quarters on partitions; one
            # PSUM bank per col-half, pipelined MMs -> copy -> half DMA-out
            for rh in range(2):          # row half of ct = out col half
                pso = ps.tile([128, 512], f32, tag=f"po{rh}")
                osb = sb.tile([128, 512], f32, tag=f"ob{rh}")
                for p in range(2):       # ct piece
                    for w in range(2):   # 512-block within piece
                        q = p * 2 + w    # psum partition quarter
                        nc.tensor.matmul(
                            pso[q * SSH:(q + 1) * SSH, :],
                            t2s[rh * 64:(rh + 1) * 64, :],
                            cts[p][rh * 64:(rh + 1) * 64, w * 512:(w + 1) * 512],
                            start=True, stop=True,
                            tile_position=(rh * 64, q * SSH),
                        )
                nc.vector.tensor_copy(osb[:], pso[:])
                nc.sync.dma_start(out_d[:, rh * 512:(rh + 1) * 512], osb[:])

    nc.compile()
    _NC_CACHE["nc"] = nc
    return nc


def _shard_inputs(x, C, U, R):
    xf = np.asarray(x, np.float32).reshape(B * S, M)
    C = np.asarray(C, np.float32)
    U = np.asarray(U, np.float32)
    R = np.asarray(R, np.float32)

    # rp[p, k*64+r] = R[r, 128k+p]
    rp = np.ascontiguousarray(
        R.reshape(RANK, KCH, 128).transpose(2, 1, 0)
    ).reshape(128, KCH * RANK)
    u2 = np.ascontiguousarray(np.concatenate([U.T, U.T], axis=0))

    in_maps = []
    for c in range(NCORES):
        i, j = divmod(c, NB)
        xs = xf[i * SSH:(i + 1) * SSH, :]
        # xp[p, k*32+s] = xs[s, 128k+p]
        xp = np.ascontiguousarray(
            xs.reshape(SSH, KCH, 128).transpose(2, 1, 0)
        ).reshape(128, KCH * SSH)
        cT = C[j * NSH:(j + 1) * NSH, :].T  # [64, 4096]
        ct = np.ascontiguousarray(
            np.concatenate([cT[:, :2048], cT[:, 2048:]], axis=0)
        )  # [128, 2048]
        in_maps.append({"xp": xp, "rp": rp, "u2": u2, "ct": ct})
    return in_maps


def _unshard_output(core_outs):
    full = np.empty((B * S, N), np.float32)
    for c in range(NCORES):
        i, j = divmod(c, NB)
        q = core_outs[c]  # [128, 1024]: q[32a+s, 512h+nr] = out[s, (4h+a)*512+nr]
        blk = q.reshape(4, SSH, 2, 512).transpose(1, 2, 0, 3).reshape(SSH, NSH)
        full[i * SSH:(i + 1) * SSH, j * NSH:(j + 1) * NSH] = blk
    return full.reshape(B, S, N)


def _ensure_ntff_hook():
    """bass_utils' axon trace path imports antenv.axon_hooks, which this
    container's antenv lacks. Register an equivalent module backed by the
    boot package's ctypes NTFF hook so trace=True (or BASS_TRACE=1) works."""
    import sys
    import types

    try:
        from antenv.axon_hooks import get_axon_ntff_profile_hook  # noqa: F401
        return
    except ImportError:
        pass
    try:
        from trn_agent_boot.trn_boot import _ntff_profile_via_ctypes

        hook = _ntff_profile_via_ctypes("/opt/axon/libaxon_pjrt.so")
    except Exception:
        hook = None
    mod = types.ModuleType("antenv.axon_hooks")
    state = {"hook": hook}
    mod.get_axon_ntff_profile_hook = lambda: state["hook"]
    mod.set_axon_ntff_profile_hook = lambda h: state.update(hook=h)
    sys.modules["antenv.axon_hooks"] = mod


def run(x, C, U, R, trace=False, **spmd_kwargs):
    from concourse.bass_utils import run_bass_kernel_spmd

    _ensure_ntff_hook()
    nc = _build_nc()
    in_maps = _shard_inputs(x, C, U, R)
    res = run_bass_kernel_spmd(
        nc, in_maps, core_ids=list(range(NCORES)), trace=trace, **spmd_kwargs
    )
    out = _unshard_output([r["out"] for r in res.results])
    return out, res


def kernel(x, C, U, R):
    out, _ = run(x, C, U, R, trace=False)
    return out


# revision 7
# speedup vs baseline: 1.1695x; 1.0140x over previous
"""Trainium2 Bass kernel for CURLoRA forward: out = x @ (C @ U @ R).T

Fused low-rank chain per core (never materializes W_approx):
  t1.T = sum_k R_k.T.T @ x_k.T   (64 K-tiles of 128, PE col-group ping-pong)
  t2.T = [U.T;U.T].T @ [t1a;t1b] (K=128 contraction absorbs the col-group sum)
  out  = col-tiled t2.T.T @ C.T  into [128,512] PSUM banks, pipelined out

Sharding (8 cores): s-dim (128 rows of x) split 4 ways x n-dim (8192 out
cols) split 2 ways. Per core DMA: 1MB x-shard + 2MB R (replicated) + 1MB
C.T shard + 0.5MB out. All transposes are host-side layout prep.
"""

import numpy as np

B, S, M, N, RANK = 2, 64, 8192, 8192, 64
NCORES = 8
SA, NB = 4, 2              # s-blocks x n-blocks = 8 cores
SSH = (B * S) // SA        # 32 s-rows per core
NSH = N // NB              # 4096 out cols per core
KCH = M // 128             # 64 contraction chunks of 128

# k-chunks per DMA piece: small first (early PE start), small last (short
# PE chase after the final bytes land)
PIECES = (8, 24, 24, 8)

_NC_CACHE = {}


def _build_nc():
    if "nc" in _NC_CACHE:
        return _NC_CACHE["nc"]
    from concourse import bacc, mybir
    import concourse.bass as bass
    from concourse.tile import TileContext, add_dep_helper

    f32 = mybir.dt.float32
    f32r = mybir.dt.float32r
    nc = bacc.Bacc(None)

    xp_d = nc.declare_dram_parameter("xp", [128, KCH * SSH], f32r, isOutput=False)
    rp_d = nc.declare_dram_parameter("rp", [128, KCH * RANK], f32r, isOutput=False)
    uq_d = nc.declare_dram_parameter("uq", [RANK, 128], f32r, isOutput=False)
    ct_d = nc.declare_dram_parameter("ct", [128, NSH // 2], f32, isOutput=False)
    out_d = nc.declare_dram_parameter("out", [128, NSH // 4], f32, isOutput=True)

    with TileContext(nc) as tc:
        with (
            tc.tile_pool(name="sb", bufs=1) as sb,
            tc.tile_pool(name="ps", bufs=1, space=bass.MemorySpace.PSUM) as ps,
        ):
            # x/R input stream; descriptor-gen split across sync/scalar
            xts, rts, xdmas, rdmas = [], [], [], []
            off = 0
            for p, kw in enumerate(PIECES):
                xt = sb.tile([128, kw * SSH], f32r, tag=f"x{p}")
                rt = sb.tile([128, kw * RANK], f32r, tag=f"r{p}")
                xdmas.append(
                    nc.sync.dma_start(xt[:], xp_d[:, off * SSH:(off + kw) * SSH])
                )
                rdmas.append(
                    nc.scalar.dma_start(rt[:], rp_d[:, off * RANK:(off + kw) * RANK])
                )
                xts.append(xt)
                rts.append(rt)
                off += kw
            uqt = sb.tile([RANK, 128], f32r, tag="uq")
            nc.gpsimd.dma_start(uqt[:], uq_d[:])
            # ct piece p holds n-cols [p*1024,(p+1)*1024) in rows 0:64 and
            # [2048+p*1024, ...) in rows 64:128. Gated mid-x-stream so the
            # critical x/R stream keeps most of the HBM bandwidth.
            cts = []
            for p in range(2):
                ct = sb.tile([128, 1024], f32, tag=f"c{p}")
                d = nc.gpsimd.dma_start(ct[:], ct_d[:, p * 1024:(p + 1) * 1024])
                add_dep_helper(
                    d.ins, (xdmas[1] if p == 0 else rdmas[1]).ins,
                    sync=True, reason="ct yields HBM bw to x/R stream",
                )
                cts.append(ct)

            # stage 1: t1.T [64, 32] x2 via col-group ping-pong, one PSUM
            # bank per group so accumulation flags don't collide
            ps1 = ps.tile([RANK, SSH], f32, tag="p1")
            k = 0
            for p, kw in enumerate(PIECES):
                for kl in range(kw):
                    nc.tensor.matmul(
                        ps1[:],
                        rts[p][:, kl * RANK:(kl + 1) * RANK],
                        xts[p][:, kl * SSH:(kl + 1) * SSH],
                        start=(k == 0), stop=(k == KCH - 1),
                    )
                    k += 1
            t1s = sb.tile([RANK, SSH], f32r, tag="t1")
            nc.vector.tensor_copy(t1s[:], ps1[:])

            # stage 2: t2.T = [U.T;U.T].T @ t1s (K=128 sums the two groups);
            # duplicated into both partition halves for stage 3's row halves
            ps2 = ps.tile([128, SSH], f32, tag="p2")
            nc.tensor.matmul(ps2[:], uqt[:], t1s[:], start=True, stop=True)
            t2s = sb.tile([128, SSH], f32, tag="t2")
            nc.vector.tensor_copy(t2s[:], ps2[:])

            # stage 3: out[s, n] col-tiled, 4 s-quarters on partitions; one
            # PSUM bank per col-half, pipelined MMs -> copy -> half DMA-out
            for rh in range(2):          # row half of ct = out col half
                pso = ps.tile([128, 512], f32, tag=f"po{rh}")
                osb = sb.tile([128, 512], f32, tag=f"ob{rh}")
                for p in range(2):       # ct piece
                    for w in range(2):   # 512-block within piece
                        q = p * 2 + w    # psum partition quarter
                        nc.tensor.matmul(
                            pso[q * SSH:(q + 1) * SSH, :],
                            t2s[rh * 64:(rh + 1) * 64, :],
                            cts[p][rh * 64:(rh + 1) * 64, w * 512:(w + 1) * 512],
                            start=True, stop=True,
                            tile_position=(rh * 64, q * SSH),
                        )
                nc.vector.tensor_copy(osb[:], pso[:])
                nc.sync.dma_start(out_d[:, rh * 512:(rh + 1) * 512], osb[:])

    nc.compile()
    _NC_CACHE["nc"] = nc
    return nc


def _shard_inputs(x, C, U, R):
    xf = np.asarray(x, np.float32).reshape(B * S, M)
    C = np.asarray(C, np.float32)
    U = np.asarray(U, np.float32)
    R = np.asarray(R, np.float32)

    # rp[p, k*64+r] = R[r, 128k+p]
    rp = np.ascontiguousarray(
        R.reshape(RANK, KCH, 128).transpose(2, 1, 0)
    ).reshape(128, KCH * RANK)
    uq = np.ascontiguousarray(np.concatenate([U.T, U.T], axis=1))

    in_maps = []
    for c in range(NCORES):
        i, j = divmod(c, NB)
        xs = xf[i * SSH:(i + 1) * SSH, :]
        # xp[p, k*32+s] = xs[s, 128k+p]
        xp = np.ascontiguousarray(
            xs.reshape(SSH, KCH, 128).transpose(2, 1, 0)
        ).reshape(128, KCH * SSH)
        cT = C[j * NSH:(j + 1) * NSH, :].T  # [64, 4096]
        ct = np.ascontiguousarray(
            np.concatenate([cT[:, :2048], cT[:, 2048:]], axis=0)
        )  # [128, 2048]
        in_maps.append({"xp": xp, "rp": rp, "uq": uq, "ct": ct})
    return in_maps


def _unshard_output(core_outs):
    full = np.empty((B * S, N), np.float32)
    for c in range(NCORES):
        i, j = divmod(c, NB)
        q = core_outs[c]  # [128, 1024]: q[32a+s, 512h+nr] = out[s, (4h+a)*512+nr]
        blk = q.reshape(4, SSH, 2, 512).transpose(1, 2, 0, 3).reshape(SSH, NSH)
        full[i * SSH:(i + 1) * SSH, j * NSH:(j + 1) * NSH] = blk
    return full.reshape(B, S, N)


def _ensure_ntff_hook():
    """bass_utils' axon trace path imports antenv.axon_hooks, which this
    container's antenv lacks. Register an equivalent module backed by the
    boot package's ctypes NTFF hook so trace=True (or BASS_TRACE=1) works."""
    import sys
    import types

    try:
        from antenv.axon_hooks import get_axon_ntff_profile_hook  # noqa: F401
        return
    except ImportError:
        pass
    try:
        from trn_agent_boot.trn_boot import _ntff_profile_via_ctypes

        hook = _ntff_profile_via_ctypes("/opt/axon/libaxon_pjrt.so")
    except Exception:
        hook = None
    mod = types.ModuleType("antenv.axon_hooks")
    state = {"hook": hook}
    mod.get_axon_ntff_profile_hook = lambda: state["hook"]
    mod.set_axon_ntff_profile_hook = lambda h: state.update(hook=h)
    sys.modules["antenv.axon_hooks"] = mod


def run(x, C, U, R, trace=False, **spmd_kwargs):
    from concourse.bass_utils import run_bass_kernel_spmd

    _ensure_ntff_hook()
    nc = _build_nc()
    in_maps = _shard_inputs(x, C, U, R)
    res = run_bass_kernel_spmd(
        nc, in_maps, core_ids=list(range(NCORES)), trace=trace, **spmd_kwargs
    )
    out = _unshard_output([r["out"] for r in res.results])
    return out, res


def kernel(x, C, U, R):
    out, _ = run(x, C, U, R, trace=False)
    return out


# revision 8
# speedup vs baseline: 1.2296x; 1.0514x over previous
"""Trainium2 Bass kernel for CURLoRA forward: out = x @ (C @ U @ R).T

Fused low-rank chain per core (never materializes the [8192, 8192] W):
  t1.T = sum_k R_k.T.T @ x_k.T     (64 K-tiles of 128, f32r, PSUM-accumulated)
  t2.T = [U.T|U.T].T @ t1.T        (one f32r matmul, M=128 duplicates t2.T
                                    into both partition halves)
  out  = t2.T.T @ C.T              (fp32, col-tiled into two [128,512] PSUM
                                    banks -> full-bandwidth output DMA)

Sharding (8 cores, no collectives): the 128 rows of x are split 4 ways and
the 8192 output columns 2 ways. Per core DMA: 1MB x-shard + 2MB R
(replicated; irreducible without cross-core comms) + 1MB C.T shard + 0.5MB
out = 4.5MB vs 6.75MB for the "shard C rows only" layout. All transposes
are host-side layout prep during sharding; every FLOP runs on-device.

Hand-scheduled raw bass (no Tile): per-DMA semaphores, engine-parallel
descriptor generation (x on sync, R on scalar, C.T/U on gpsimd), C.T gated
behind the x/R stream so the stage-1-critical bytes keep the HBM bandwidth,
and a pipelined matmul->copy->DMA output tail per PSUM bank.
"""

import numpy as np

B, S, M, N, RANK = 2, 64, 8192, 8192, 64
NCORES = 8
SA, NB = 4, 2              # s-blocks x n-blocks = 8 cores
SSH = (B * S) // SA        # 32 s-rows per core
NSH = N // NB              # 4096 out cols per core
KCH = M // 128             # 64 contraction chunks of 128

# k-chunks per x/R DMA piece: small first piece starts the PE early, small
# last piece keeps the post-stream PE chase short
PIECES = (8, 24, 24, 8)

_NC_CACHE = {}


def _build_nc():
    if "nc" in _NC_CACHE:
        return _NC_CACHE["nc"]
    from contextlib import ExitStack
    from concourse import mybir
    import concourse.bass as bass

    f32 = mybir.dt.float32
    f32r = mybir.dt.float32r
    nc = bass.Bass()

    xp_d = nc.declare_dram_parameter("xp", [128, KCH * SSH], f32r, isOutput=False)
    rp_d = nc.declare_dram_parameter("rp", [128, KCH * RANK], f32r, isOutput=False)
    uq_d = nc.declare_dram_parameter("uq", [RANK, 128], f32r, isOutput=False)
    ct_d = nc.declare_dram_parameter("ct", [128, NSH // 2], f32, isOutput=False)
    out_d = nc.declare_dram_parameter("out", [128, NSH // 4], f32, isOutput=True)

    ctx = ExitStack()
    with ctx:
        xts = [
            ctx.enter_context(nc.sbuf_tensor(f"xt{i}", [128, kw * SSH], f32r))
            for i, kw in enumerate(PIECES)
        ]
        rts = [
            ctx.enter_context(nc.sbuf_tensor(f"rt{i}", [128, kw * RANK], f32r))
            for i, kw in enumerate(PIECES)
        ]
        uqt = ctx.enter_context(nc.sbuf_tensor("uqt", [RANK, 128], f32r))
        cts = [
            ctx.enter_context(nc.sbuf_tensor(f"ct{i}", [128, 1024], f32))
            for i in range(2)
        ]
        t1s = ctx.enter_context(nc.sbuf_tensor("t1s", [RANK, SSH], f32r))
        t2s = ctx.enter_context(nc.sbuf_tensor("t2s", [128, SSH], f32))
        osbs = [
            ctx.enter_context(nc.sbuf_tensor(f"osb{i}", [128, 512], f32))
            for i in range(2)
        ]
        # one PSUM bank each ([128, 512] f32 = exactly one bank)
        ps1 = ctx.enter_context(nc.psum_tensor("ps1", [128, 512], f32))
        ps2 = ctx.enter_context(nc.psum_tensor("ps2", [128, 512], f32))
        psos = [
            ctx.enter_context(nc.psum_tensor(f"pso{i}", [128, 512], f32))
            for i in range(2)
        ]

        # one semaphore per DMA: queue completions of distinct DMAs are not
        # ordered, so a shared counter would be unsound
        sxs = [ctx.enter_context(nc.semaphore(f"sx{i}")) for i in range(len(PIECES))]
        srs = [ctx.enter_context(nc.semaphore(f"sr{i}")) for i in range(len(PIECES))]
        scu = ctx.enter_context(nc.semaphore("scu"))
        scs = [ctx.enter_context(nc.semaphore(f"sc{i}")) for i in range(2)]
        sm = ctx.enter_context(nc.semaphore("sm"))
        sv = ctx.enter_context(nc.semaphore("sv"))
        sos = [ctx.enter_context(nc.semaphore(f"so{i}")) for i in range(2)]

        block = ctx.enter_context(nc.Block())

        @block.sync
        def _(sync):
            off = 0
            for p, kw in enumerate(PIECES):
                sync.dma_start(
                    xts[p][:], xp_d[:, off * SSH:(off + kw) * SSH]
                ).then_inc(sxs[p], 16)
                off += kw
            sync.wait_ge(sv, 3)
            sync.dma_start(out_d[:, 0:512], osbs[0][:]).then_inc(sos[0], 16)
            sync.wait_ge(sv, 4)
            sync.dma_start(out_d[:, 512:1024], osbs[1][:]).then_inc(sos[1], 16)
            sync.wait_ge(sos[0], 16)
            sync.wait_ge(sos[1], 16)

        @block.scalar
        def _(scalar):
            off = 0
            for p, kw in enumerate(PIECES):
                scalar.dma_start(
                    rts[p][:], rp_d[:, off * RANK:(off + kw) * RANK]
                ).then_inc(srs[p], 16)
                off += kw

        @block.gpsimd
        def _(g):
            g.dma_start(uqt[:], uq_d[:]).then_inc(scu, 16)
            g.wait_ge(sxs[1], 16)  # let the x/R stream lead on HBM bw
            g.dma_start(cts[0][:], ct_d[:, 0:1024]).then_inc(scs[0], 16)
            g.dma_start(cts[1][:], ct_d[:, 1024:2048]).then_inc(scs[1], 16)

        @block.tensor
        def _(t):
            k = 0
            last_mm = None
            for p, kw in enumerate(PIECES):
                t.wait_ge(sxs[p], 16)
                t.wait_ge(srs[p], 16)
                for kl in range(kw):
                    last_mm = nc.tensor.matmul(
                        ps1[0:RANK, 0:SSH],
                        rts[p][:, kl * RANK:(kl + 1) * RANK],
                        xts[p][:, kl * SSH:(kl + 1) * SSH],
                        start=(k == 0), stop=(k == KCH - 1),
                    )
                    k += 1
            last_mm.then_inc(sm, 1)                      # sm=1: stage 1 done
            t.wait_ge(sv, 1)                             # t1s copied
            t.wait_ge(scu, 16)                           # uqt loaded
            nc.tensor.matmul(ps2[:, 0:SSH], uqt[:], t1s[:],
                             start=True, stop=True).then_inc(sm, 1)  # sm=2
            t.wait_ge(sv, 2)                             # t2s copied
            t.wait_ge(scs[0], 16)                        # ct loaded
            t.wait_ge(scs[1], 16)
            for rh in range(2):                          # psum bank = col half
                last_mm = None
                for p in range(2):
                    for w in range(2):
                        q = p * 2 + w                    # psum partition quarter
                        last_mm = nc.tensor.matmul(
                            psos[rh][q * SSH:(q + 1) * SSH, :],
                            t2s[rh * 64:(rh + 1) * 64, :],
                            cts[p][rh * 64:(rh + 1) * 64, w * 512:(w + 1) * 512],
                            start=True, stop=True,
                            tile_position=(rh * 64, q * SSH),
                        )
                last_mm.then_inc(sm, 1)                  # sm=3, sm=4

        @block.vector
        def _(v):
            v.wait_ge(sm, 1)
            nc.vector.tensor_copy(t1s[:], ps1[0:RANK, 0:SSH]).then_inc(sv, 1)
            v.wait_ge(sm, 2)
            nc.vector.tensor_copy(t2s[:], ps2[:, 0:SSH]).then_inc(sv, 1)
            v.wait_ge(sm, 3)
            nc.vector.tensor_copy(osbs[0][:], psos[0][:]).then_inc(sv, 1)
            v.wait_ge(sm, 4)
            nc.vector.tensor_copy(osbs[1][:], psos[1][:]).then_inc(sv, 1)

    _NC_CACHE["nc"] = nc
    return nc


def _shard_inputs(x, C, U, R):
    xf = np.asarray(x, np.float32).reshape(B * S, M)
    C = np.asarray(C, np.float32)
    U = np.asarray(U, np.float32)
    R = np.asarray(R, np.float32)

    # rp[p, k*64+r] = R[r, 128k+p]
    rp = np.ascontiguousarray(
        R.reshape(RANK, KCH, 128).transpose(2, 1, 0)
    ).reshape(128, KCH * RANK)
    # uq = U.T duplicated along columns: stage 2's lhsT, M=128 so t2.T lands
    # duplicated in both partition halves (stage 3 reads them as row halves)
    uq = np.ascontiguousarray(np.concatenate([U.T, U.T], axis=1))

    in_maps = []
    for c in range(NCORES):
        i, j = divmod(c, NB)
        xs = xf[i * SSH:(i + 1) * SSH, :]
        # xp[p, k*32+s] = xs[s, 128k+p]
        xp = np.ascontiguousarray(
            xs.reshape(SSH, KCH, 128).transpose(2, 1, 0)
        ).reshape(128, KCH * SSH)
        # ct rows 0:64 = C.T cols [0,2048) of this n-shard, rows 64:128 =
        # cols [2048,4096) -- full 128-partition (= full-bandwidth) DMA
        cT = C[j * NSH:(j + 1) * NSH, :].T  # [64, 4096]
        ct = np.ascontiguousarray(
            np.concatenate([cT[:, :2048], cT[:, 2048:]], axis=0)
        )  # [128, 2048]
        in_maps.append({"xp": xp, "rp": rp, "uq": uq, "ct": ct})
    return in_maps


def _unshard_output(core_outs):
    full = np.empty((B * S, N), np.float32)
    for c in range(NCORES):
        i, j = divmod(c, NB)
        q = core_outs[c]  # [128, 1024]: q[32a+s, 512h+nr] = out[s, (4h+a)*512+nr]
        blk = q.reshape(4, SSH, 2, 512).transpose(1, 2, 0, 3).reshape(SSH, NSH)
        full[i * SSH:(i + 1) * SSH, j * NSH:(j + 1) * NSH] = blk
    return full.reshape(B, S, N)


def _ensure_ntff_hook():
    """bass_utils' axon trace path imports antenv.axon_hooks, which this
    container's antenv lacks. Register an equivalent module backed by the
    boot package's ctypes NTFF hook so trace=True (or BASS_TRACE=1) works."""
    import sys
    import types

    try:
        from antenv.axon_hooks import get_axon_ntff_profile_hook  # noqa: F401
        return
    except ImportError:
        pass
    try:
        from trn_agent_boot.trn_boot import _ntff_profile_via_ctypes

        hook = _ntff_profile_via_ctypes("/opt/axon/libaxon_pjrt.so")
    except Exception:
        hook = None
    mod = types.ModuleType("antenv.axon_hooks")
    state = {"hook": hook}
    mod.get_axon_ntff_profile_hook = lambda: state["hook"]
    mod.set_axon_ntff_profile_hook = lambda h: state.update(hook=h)
    sys.modules["antenv.axon_hooks"] = mod


def run(x, C, U, R, trace=False, **spmd_kwargs):
    from concourse.bass_utils import run_bass_kernel_spmd

    _ensure_ntff_hook()
    nc = _build_nc()
    in_maps = _shard_inputs(x, C, U, R)
    res = run_bass_kernel_spmd(
        nc, in_maps, core_ids=list(range(NCORES)), trace=trace, **spmd_kwargs
    )
    out = _unshard_output([r["out"] for r in res.results])
    return out, res


def kernel(x, C, U, R):
    out, _ = run(x, C, U, R, trace=False)
    return out


# revision 12
# speedup vs baseline: 1.2415x; 1.0096x over previous
"""Trainium2 Bass kernel for CURLoRA forward: out = x @ (C @ U @ R).T

Fused low-rank chain per core (never materializes the [8192, 8192] W):
  t1.T = sum_k R_k.T.T @ x_k.T     (64 K-tiles of 128, f32r, PSUM-accumulated)
  t2.T = [U.T|U.T].T @ t1.T        (one f32r matmul, M=128 duplicates t2.T
                                    into both partition halves)
  out  = t2.T.T @ C.T              (fp32, col-tiled into two [128,512] PSUM
                                    banks -> full-bandwidth output DMA)

Sharding (8 cores, no collectives): the 128 rows of x are split 4 ways and
the 8192 output columns 2 ways. Per core DMA: 1MB x-shard + 2MB R
(replicated; irreducible without cross-core comms) + 1MB C.T shard + 0.5MB
out = 4.5MB vs 6.75MB for the "shard C rows only" layout. All transposes
are host-side layout prep during sharding; every FLOP runs on-device.

Hand-scheduled raw bass (no Tile): per-DMA semaphores, engine-parallel
descriptor generation (x on sync, R on scalar, C.T/U on gpsimd), C.T gated
behind the x/R stream so the stage-1-critical bytes keep the HBM bandwidth,
and a pipelined matmul->copy->DMA output tail per PSUM bank.
"""

import numpy as np

B, S, M, N, RANK = 2, 64, 8192, 8192, 64
NCORES = 8
SA, NB = 4, 2              # s-blocks x n-blocks = 8 cores
SSH = (B * S) // SA        # 32 s-rows per core
NSH = N // NB              # 4096 out cols per core
KCH = M // 128             # 64 contraction chunks of 128

# k-chunks per x/R DMA piece: small first piece starts the PE early, small
# last piece keeps the post-stream PE chase short
PIECES = (8, 24, 24, 8)

_NC_CACHE = {}


def _build_nc():
    if "nc" in _NC_CACHE:
        return _NC_CACHE["nc"]
    from contextlib import ExitStack
    from concourse import mybir
    import concourse.bass as bass

    f32 = mybir.dt.float32
    f32r = mybir.dt.float32r
    nc = bass.Bass()

    xp_d = nc.declare_dram_parameter("xp", [128, KCH * SSH], f32r, isOutput=False)
    rp_d = nc.declare_dram_parameter("rp", [128, KCH * RANK], f32r, isOutput=False)
    uq_d = nc.declare_dram_parameter("uq", [RANK, 128], f32r, isOutput=False)
    ct_d = nc.declare_dram_parameter("ct", [128, NSH // 2], f32, isOutput=False)
    out_d = nc.declare_dram_parameter("out", [128, NSH // 4], f32, isOutput=True)

    ctx = ExitStack()
    with ctx:
        xts = [
            ctx.enter_context(nc.sbuf_tensor(f"xt{i}", [128, kw * SSH], f32r))
            for i, kw in enumerate(PIECES)
        ]
        rts = [
            ctx.enter_context(nc.sbuf_tensor(f"rt{i}", [128, kw * RANK], f32r))
            for i, kw in enumerate(PIECES)
        ]
        uqt = ctx.enter_context(nc.sbuf_tensor("uqt", [RANK, 128], f32r))
        cts = [
            ctx.enter_context(nc.sbuf_tensor(f"ct{i}", [128, 1024], f32))
            for i in range(2)
        ]
        t1s = ctx.enter_context(nc.sbuf_tensor("t1s", [RANK, SSH], f32r))
        t2s = ctx.enter_context(nc.sbuf_tensor("t2s", [128, SSH], f32))
        osbs = [
            ctx.enter_context(nc.sbuf_tensor(f"osb{i}", [128, 512], f32))
            for i in range(2)
        ]
        # one PSUM bank each ([128, 512] f32 = exactly one bank)
        ps1 = ctx.enter_context(nc.psum_tensor("ps1", [128, 512], f32))
        ps2 = ctx.enter_context(nc.psum_tensor("ps2", [128, 512], f32))
        psos = [
            ctx.enter_context(nc.psum_tensor(f"pso{i}", [128, 512], f32))
            for i in range(2)
        ]

        # one semaphore per DMA: queue completions of distinct DMAs are not
        # ordered, so a shared counter would be unsound
        sxs = [ctx.enter_context(nc.semaphore(f"sx{i}")) for i in range(len(PIECES))]
        srs = [ctx.enter_context(nc.semaphore(f"sr{i}")) for i in range(len(PIECES))]
        scu = ctx.enter_context(nc.semaphore("scu"))
        scs = [ctx.enter_context(nc.semaphore(f"sc{i}")) for i in range(2)]
        sm = ctx.enter_context(nc.semaphore("sm"))
        sv = ctx.enter_context(nc.semaphore("sv"))
        sos = [ctx.enter_context(nc.semaphore(f"so{i}")) for i in range(2)]

        block = ctx.enter_context(nc.Block())

        @block.sync
        def _(sync):
            off = 0
            for p, kw in enumerate(PIECES):
                sync.dma_start(
                    xts[p][:], xp_d[:, off * SSH:(off + kw) * SSH]
                ).then_inc(sxs[p], 16)
                off += kw
            sync.wait_ge(sv, 3)
            sync.dma_start(out_d[:, 0:512], osbs[0][:]).then_inc(sos[0], 16)
            sync.wait_ge(sv, 4)
            sync.dma_start(out_d[:, 512:1024], osbs[1][:]).then_inc(sos[1], 16)
            # no completion wait: the block-end drain + NEFF teardown give the
            # 0.5MB transfer far more slack than its ~1.5us drain time

        @block.scalar
        def _(scalar):
            off = 0
            for p, kw in enumerate(PIECES):
                scalar.dma_start(
                    rts[p][:], rp_d[:, off * RANK:(off + kw) * RANK]
                ).then_inc(srs[p], 16)
                off += kw

        @block.gpsimd
        def _(g):
            g.dma_start(uqt[:], uq_d[:]).then_inc(scu, 16)
            g.wait_ge(sxs[1], 16)  # let the x/R stream lead on HBM bw
            g.dma_start(cts[0][:], ct_d[:, 0:1024]).then_inc(scs[0], 16)
            g.dma_start(cts[1][:], ct_d[:, 1024:2048]).then_inc(scs[1], 16)

        @block.tensor
        def _(t):
            k = 0
            last_mm = None
            for p, kw in enumerate(PIECES):
                t.wait_ge(sxs[p], 16)
                t.wait_ge(srs[p], 16)
                for kl in range(kw):
                    last_mm = nc.tensor.matmul(
                        ps1[0:RANK, 0:SSH],
                        rts[p][:, kl * RANK:(kl + 1) * RANK],
                        xts[p][:, kl * SSH:(kl + 1) * SSH],
                        start=(k == 0), stop=(k == KCH - 1),
                    )
                    k += 1
            last_mm.then_inc(sm, 1)                      # sm=1: stage 1 done
            t.wait_ge(sv, 1)                             # t1s copied
            t.wait_ge(scu, 16)                           # uqt loaded
            nc.tensor.matmul(ps2[:, 0:SSH], uqt[:], t1s[:],
                             start=True, stop=True).then_inc(sm, 1)  # sm=2
            t.wait_ge(sv, 2)                             # t2s copied
            t.wait_ge(scs[0], 16)                        # ct loaded
            t.wait_ge(scs[1], 16)
            for rh in range(2):                          # psum bank = col half
                last_mm = None
                for p in range(2):
                    for w in range(2):
                        q = p * 2 + w                    # psum partition quarter
                        last_mm = nc.tensor.matmul(
                            psos[rh][q * SSH:(q + 1) * SSH, :],
                            t2s[rh * 64:(rh + 1) * 64, :],
                            cts[p][rh * 64:(rh + 1) * 64, w * 512:(w + 1) * 512],
                            start=True, stop=True,
                            tile_position=(rh * 64, q * SSH),
                        )
                last_mm.then_inc(sm, 1)                  # sm=3, sm=4

        @block.vector
        def _(v):
            v.wait_ge(sm, 1)
            nc.vector.tensor_copy(t1s[:], ps1[0:RANK, 0:SSH]).then_inc(sv, 1)
            v.wait_ge(sm, 2)
            nc.vector.tensor_copy(t2s[:], ps2[:, 0:SSH]).then_inc(sv, 1)
            v.wait_ge(sm, 3)
            nc.vector.tensor_copy(osbs[0][:], psos[0][:]).then_inc(sv, 1)
            v.wait_ge(sm, 4)
            nc.vector.tensor_copy(osbs[1][:], psos[1][:]).then_inc(sv, 1)

    _NC_CACHE["nc"] = nc
    return nc


def _shard_inputs(x, C, U, R):
    xf = np.asarray(x, np.float32).reshape(B * S, M)
    C = np.asarray(C, np.float32)
    U = np.asarray(U, np.float32)
    R = np.asarray(R, np.float32)

    # rp[p, k*64+r] = R[r, 128k+p]
    rp = np.ascontiguousarray(
        R.reshape(RANK, KCH, 128).transpose(2, 1, 0)
    ).reshape(128, KCH * RANK)
    # uq = U.T duplicated along columns: stage 2's lhsT, M=128 so t2.T lands
    # duplicated in both partition halves (stage 3 reads them as row halves)
    uq = np.ascontiguousarray(np.concatenate([U.T, U.T], axis=1))

    in_maps = []
    for c in range(NCORES):
        i, j = divmod(c, NB)
        xs = xf[i * SSH:(i + 1) * SSH, :]
        # xp[p, k*32+s] = xs[s, 128k+p]
        xp = np.ascontiguousarray(
            xs.reshape(SSH, KCH, 128).transpose(2, 1, 0)
        ).reshape(128, KCH * SSH)
        # ct rows 0:64 = C.T cols [0,2048) of this n-shard, rows 64:128 =
        # cols [2048,4096) -- full 128-partition (= full-bandwidth) DMA
        cT = C[j * NSH:(j + 1) * NSH, :].T  # [64, 4096]
        ct = np.ascontiguousarray(
            np.concatenate([cT[:, :2048], cT[:, 2048:]], axis=0)
        )  # [128, 2048]
        in_maps.append({"xp": xp, "rp": rp, "uq": uq, "ct": ct})
    return in_maps


def _unshard_output(core_outs):
    full = np.empty((B * S, N), np.float32)
    for c in range(NCORES):
        i, j = divmod(c, NB)
        q = core_outs[c]  # [128, 1024]: q[32a+s, 512h+nr] = out[s, (4h+a)*512+nr]
        blk = q.reshape(4, SSH, 2, 512).transpose(1, 2, 0, 3).reshape(SSH, NSH)
        full[i * SSH:(i + 1) * SSH, j * NSH:(j + 1) * NSH] = blk
    return full.reshape(B, S, N)


def _ensure_ntff_hook():
    """bass_utils' axon trace path imports antenv.axon_hooks, which this
    container's antenv lacks. Register an equivalent module backed by the
    boot package's ctypes NTFF hook so trace=True (or BASS_TRACE=1) works."""
    import sys
    import types

    try:
        from antenv.axon_hooks import get_axon_ntff_profile_hook  # noqa: F401
        return
    except ImportError:
        pass
    try:
        from trn_agent_boot.trn_boot import _ntff_profile_via_ctypes

        hook = _ntff_profile_via_ctypes("/opt/axon/libaxon_pjrt.so")
    except Exception:
        hook = None
    mod = types.ModuleType("antenv.axon_hooks")
    state = {"hook": hook}
    mod.get_axon_ntff_profile_hook = lambda: state["hook"]
    mod.set_axon_ntff_profile_hook = lambda h: state.update(hook=h)
    sys.modules["antenv.axon_hooks"] = mod


def run(x, C, U, R, trace=False, **spmd_kwargs):
    from concourse.bass_utils import run_bass_kernel_spmd

    _ensure_ntff_hook()
    nc = _build_nc()
    in_maps = _shard_inputs(x, C, U, R)
    res = run_bass_kernel_spmd(
        nc, in_maps, core_ids=list(range(NCORES)), trace=trace, **spmd_kwargs
    )
    out = _unshard_output([r["out"] for r in res.results])
    return out, res


def kernel(x, C, U, R):
    out, _ = run(x, C, U, R, trace=False)
    return out
